# revision 1
# baseline (speedup 1.0000x reference)
"""RWKV6 block (nn_Block_14602888806424) on 8 Trainium2 NeuronCores.

Token-sharded (sequence-parallel): each core owns 512 tokens (B=2 x 4
blocks); matmuls/LNs/mixing are token-local in channel-major layout.
r/k/w/v are redistributed head-sharded via AllToAll around the chunked
(L=128) WKV linear-attention scan (4 heads/core, f32); GroupNorm is
head-local; a second AllToAll returns gn(y). A small AllGather carries the
1-token boundary halo for the second token-shift. Projections/FFN in bf16
with f32 PSUM accumulation; weights host-pre-tiled into contiguous
128-column blocks for efficient DMA streaming.
"""

import sys
import numpy as np

sys.path.insert(0, "/opt/trn_rl_repo")

import concourse.bass as bass
import concourse.bacc as bacc
import concourse.mybir as mybir
import concourse.tile as tile
from concourse import bass_utils

F32 = mybir.dt.float32
BF16 = mybir.dt.bfloat16
F8 = mybir.dt.float8e4
NP_BF16 = mybir.dt.np(BF16)
NP_F8 = mybir.dt.np(F8)
AF = mybir.ActivationFunctionType
ALU = mybir.AluOpType

B, T, C, H, N, FF = 2, 2048, 2048, 32, 64, 7168
D_MIX, D_DECAY = 32, 64
EPS_LN = 1e-5
EPS_LNX = 1e-5 * 8.0**2
NCORE = 8
TB = 512
KC = C // 128          # 16
KF = FF // 128         # 56
LCH = 256              # channels per core (4 heads)
RG = [list(range(NCORE))]
F8S = 8.0              # fp8 scale on each ffn operand


def build_program():
    nc = bacc.Bacc("TRN2", target_bir_lowering=False, debug=False,
                   num_devices=NCORE, enable_asserts=False)

    def din(name, shape, dt=F32):
        return nc.dram_tensor(name, list(shape), dt, kind="ExternalInput").ap()

    xT = din("xT", (C, TB + 1))
    halo_mask = din("halo_mask", (128, 1))
    sel_prev = din("sel_prev", (NCORE, 1), BF16)
    u_loc = din("u_loc", (128, 2))
    lnx_bc = din("lnx_bc", (128, 2 * LCH))
    ln1_wb = din("ln1_wb", (C, 2))
    ln2_wb = din("ln2_wb", (C, 2))
    tm_maaT = din("tm_maaT", (C, 6))
    cm_maaT = din("cm_maaT", (C, 2))
    td_col = din("td_col", (C, 1))
    ident = din("ident", (128, 128))
    mask_su = din("mask_su", (128, 128))
    maa_w1 = din("maa_w1", (C, 5 * D_MIX), BF16)
    maa_w2p = din("maa_w2p", (32, 5, KC, 128), BF16)
    td_w1 = din("td_w1", (C, D_DECAY), BF16)
    td_w2p = din("td_w2p", (64, KC, 128), BF16)
    Wp = {k: din(f"W{k}_p", (KC * C, 128), BF16)
          for k in ["r", "k", "g", "o", "cr"]}
    Wv = din("Wv", (C, C), BF16)
    Wck_p = din("Wck_p", (KF * C, 128), BF16)
    Wcv_p = din("Wcv_p", (KC * FF, 128), BF16)

    outT = nc.dram_tensor("out", [C, TB], F32, kind="ExternalOutput").ap()

    with tile.TileContext(nc) as tc:
        import contextlib
        with contextlib.ExitStack() as ctx:
            dram = ctx.enter_context(tc.tile_pool(name="dram", bufs=1,
                                                  space="DRAM"))
            cpool = ctx.enter_context(tc.tile_pool(name="const", bufs=1))
            big = ctx.enter_context(tc.tile_pool(name="big", bufs=1))
            wstr = ctx.enter_context(tc.tile_pool(name="wstr", bufs=3))
            sc = ctx.enter_context(tc.tile_pool(name="scratch", bufs=2))
            scw = ctx.enter_context(tc.tile_pool(name="scw", bufs=1))
            lnp = ctx.enter_context(tc.tile_pool(name="lnp", bufs=1))
            ps = ctx.enter_context(
                tc.tile_pool(name="psum", bufs=8, space="PSUM"))

            def pp(p_, f_):
                return ps.tile([p_, f_], F32, tag="pp", name="pp")

            # ---- DRAM internals ----
            a2a_in = dram.tile([NCORE, 3, LCH, TB], F32, tag="a2a_in")
            a2a_out = dram.tile([NCORE, 3, LCH, TB], F32, tag="a2a_out")
            a2v_in = dram.tile([NCORE, TB, LCH], F32, tag="a2v_in")
            a2v_out = dram.tile([NCORE, TB, LCH], F32, tag="a2v_out")
            a2b_in = dram.tile([NCORE, LCH, TB], F32, tag="a2b_in")
            a2b_out = dram.tile([NCORE, LCH, TB], F32, tag="a2b_out")
            ag_in = dram.tile([1, C], BF16, tag="ag_in")
            ag_out = dram.tile([NCORE, C], BF16, tag="ag_out",
                               addr_space="Shared")
            x2d = dram.tile([C, TB + 1], F32, tag="x2d")

            # ---- constants ----
            def cload(name, src, shape, dt=F32, rearr=None):
                t = cpool.tile(list(shape), dt, tag=name)
                nc.sync.dma_start(t[:], src if rearr is None
                                  else src.rearrange(rearr, p=128))
                return t

            c_ln1 = cload("c_ln1", ln1_wb, (128, KC, 2), F32, "(k p) f -> p k f")
            c_ln2 = cload("c_ln2", ln2_wb, (128, KC, 2), F32, "(k p) f -> p k f")
            c_tm = cload("c_tm", tm_maaT, (128, KC, 6), F32, "(k p) f -> p k f")
            c_cm = cload("c_cm", cm_maaT, (128, KC, 2), F32, "(k p) f -> p k f")
            c_td = cload("c_td", td_col, (128, KC, 1), F32, "(k p) f -> p k f")
            c_hm = cload("c_hm", halo_mask, (128, 1))
            c_sel = cload("c_sel", sel_prev, (NCORE, 1), BF16)
            c_u = cload("c_u", u_loc, (128, 2))
            c_lnx = cload("c_lnx", lnx_bc, (128, 2 * LCH))
            c_id = cload("c_id", ident, (128, 128))
            c_msk = cload("c_msk", mask_su, (128, 128))
            c_w1 = cload("c_w1", maa_w1, (128, KC, 5 * D_MIX), BF16,
                         "(k p) f -> p k f")
            c_td1 = cload("c_td1", td_w1, (128, KC, D_DECAY), BF16,
                          "(k p) f -> p k f")
            c_td2 = cload("c_td2", td_w2p, (64, KC, 128), BF16)
            ones_col = cpool.tile([128, 1], F32, tag="ones_col")
            nc.vector.memset(ones_col[:], 1.0)
            ones_row = cpool.tile([1, 128], F32, tag="ones_row")
            nc.vector.memset(ones_row[:], 1.0)
            for _cv in (EPS_LN, EPS_LNX):
                cvt = cpool.tile([128, 1], F32, tag=f"cv{_cv}", name="cvt")
                nc.vector.memset(cvt[:], _cv)
                nc.const_aps.aps[(F32, _cv)] = cvt[:]

            # ---- persistent SBUF ----
            ht = big.tile([128, KC, TB + 1], BF16, tag="ht")
            xx = big.tile([128, KC, TB], BF16, tag="xx")      # later xk2
            gsb = big.tile([128, KC, TB], BF16, tag="gsb")    # later xr2

            # ============ layernorm over TB+1 cols ============
            def layer_norm_ext(src_fn, dst_view, wb, eps):
                """src_fn(k)->(128,TB+1) f32 AP-producing fn; called twice."""
                psA, psB = pp(1, TB), pp(1, 1)
                psA2, psB2 = pp(1, TB), pp(1, 1)
                for k in range(KC):
                    s = src_fn(k)
                    sq = sc.tile([128, TB + 1], F32, tag="e2")
                    nc.scalar.activation(sq[:], s[:], AF.Square)
                    st, sp = (k == 0), (k == KC - 1)
                    nc.tensor.matmul(psA[:], ones_col[:], s[:, 0:TB],
                                     start=st, stop=sp)
                    nc.tensor.matmul(psB[:], ones_col[:], s[:, TB:TB + 1],
                                     start=st, stop=sp)
                    nc.tensor.matmul(psA2[:], ones_col[:], sq[:, 0:TB],
                                     start=st, stop=sp)
                    nc.tensor.matmul(psB2[:], ones_col[:], sq[:, TB:TB + 1],
                                     start=st, stop=sp)
                stats = lnp.tile([1, 2 * (TB + 1)], F32, tag="ln_stats")
                mean, msq = stats[:, 0:TB + 1], stats[:, TB + 1:]
                nc.scalar.activation(mean[:, 0:TB], psA[:], AF.Copy,
                                     scale=1.0 / C)
                nc.scalar.activation(mean[:, TB:TB + 1], psB[:], AF.Copy,
                                     scale=1.0 / C)
                nc.scalar.activation(msq[:, 0:TB], psA2[:], AF.Copy,
                                     scale=1.0 / C)
                nc.scalar.activation(msq[:, TB:TB + 1], psB2[:], AF.Copy,
                                     scale=1.0 / C)
                wk = lnp.tile([1, TB + 1], F32, tag="ln_work")
                nc.vector.tensor_mul(wk[:], mean[:], mean[:])
                nc.vector.tensor_sub(wk[:], msq[:], wk[:])
                nc.scalar.activation(wk[:], wk[:], AF.Sqrt, bias=eps)
                nc.vector.reciprocal(wk[:], wk[:])        # inv-std per token
                bmp, bmp2 = pp(128, TB), pp(128, 1)
                bip, bip2 = pp(128, TB), pp(128, 1)
                nc.tensor.matmul(bmp[:], ones_row[:], mean[:, 0:TB],
                                 start=True, stop=True)
                nc.tensor.matmul(bmp2[:], ones_row[:], mean[:, TB:TB + 1],
                                 start=True, stop=True)
                nc.tensor.matmul(bip[:], ones_row[:], wk[:, 0:TB],
                                 start=True, stop=True)
                nc.tensor.matmul(bip2[:], ones_row[:], wk[:, TB:TB + 1],
                                 start=True, stop=True)
                bc = lnp.tile([128, 2 * (TB + 1)], BF16, tag="ln_bc")
                bm, bi = bc[:, 0:TB + 1], bc[:, TB + 1:]
                nc.vector.tensor_copy(bm[:, 0:TB], bmp[:])
                nc.vector.tensor_copy(bm[:, TB:TB + 1], bmp2[:])
                nc.vector.tensor_copy(bi[:, 0:TB], bip[:])
                nc.vector.tensor_copy(bi[:, TB:TB + 1], bip2[:])
                for k in range(KC):
                    s = src_fn(k)
                    t = sc.tile([128, TB + 1], F32, tag="e2")
                    nc.vector.tensor_sub(t[:], s[:], bm[:])
                    nc.vector.tensor_mul(t[:], t[:], bi[:])
                    d = dst_view(k)
                    nc.vector.tensor_scalar(d, t[:], wb[:, k, 0:1],
                                            wb[:, k, 1:2], ALU.mult, ALU.add)
                    nc.vector.tensor_scalar(d[:, 0:1], d[:, 0:1], c_hm[:],
                                            None, ALU.mult)

            def xt_src(k):
                t = sc.tile([128, TB + 1], F32, tag="e1", bufs=2)
                nc.sync.dma_start(t[:], xT[128 * k:128 * (k + 1), :])
                return t

            # ============ phase A: ln1 + shift ============
            layer_norm_ext(xt_src, lambda k: ht[:, k, :], c_ln1, EPS_LN)
            for k in range(KC):
                nc.vector.tensor_sub(xx[:, k, :], ht[:, k, 0:TB],
                                     ht[:, k, 1:TB + 1])

            # ============ phase B: maa ============
            aps1, aps2 = pp(128, TB), pp(32, TB)
            for k in range(KC):
                xxx = sc.tile([128, TB], BF16, tag="xxx")
                nc.vector.scalar_tensor_tensor(
                    xxx[:], xx[:, k, :], c_tm[:, k, 0:1], ht[:, k, 1:TB + 1],
                    ALU.mult, ALU.add)
                nc.tensor.matmul(aps1[:], c_w1[:, k, 0:128], xxx[:],
                                 start=(k == 0), stop=(k == KC - 1))
                nc.tensor.matmul(aps2[:], c_w1[:, k, 128:160], xxx[:],
                                 start=(k == 0), stop=(k == KC - 1))
            aTs = [cpool.tile([32, TB], BF16, tag=f"aT{i}", name="aTs")
                   for i in range(5)]
            for i in range(4):
                nc.scalar.activation(aTs[i][:], aps1[32 * i:32 * (i + 1), :],
                                     AF.Tanh)
            nc.scalar.activation(aTs[4][:], aps2[0:32, :], AF.Tanh)

            def a_slice(i):
                return aTs[i][:]

            def make_mix(i, tag):
                mt = big.tile([128, KC, TB], BF16, tag=tag, name="mixbuf")
                for k in range(KC):
                    w2s = wstr.tile([32, 128], BF16, tag="w2s")
                    nc.sync.dma_start(w2s[:], maa_w2p[:, i, k, :])
                    mp = pp(128, TB)
                    nc.tensor.matmul(mp[:], w2s[:], a_slice(i),
                                     start=True, stop=True)
                    t = sc.tile([128, TB], F32, tag="g1")
                    nc.vector.scalar_tensor_tensor(
                        t[:], mp[:], c_tm[:, k, i + 1:i + 2], xx[:, k, :],
                        ALU.add, ALU.mult)
                    nc.vector.tensor_add(mt[:, k, :], t[:],
                                         ht[:, k, 1:TB + 1])
                return mt

            def proj_cm(wp_ap, sink, src_view):
                for m in range(KC):
                    wt = wstr.tile([128, KC, 128], BF16, tag="wstream", bufs=3)
                    nc.sync.dma_start(
                        wt[:], wp_ap[m * C:(m + 1) * C, :]
                        .rearrange("(k p) f -> p k f", p=128))
                    pt = pp(128, TB)
                    for k in range(KC):
                        nc.tensor.matmul(pt[:], wt[:, k, :], src_view(k),
                                         start=(k == 0), stop=(k == KC - 1))
                    sink(m, pt)

            def sink_a2a(idx):
                def s(m, pt):
                    st = sc.tile([128, TB], F32, tag="g2")
                    nc.vector.tensor_copy(st[:], pt[:])
                    nc.sync.dma_start(
                        a2a_in[m // 2, idx,
                               128 * (m % 2):128 * (m % 2) + 128, :], st[:])
                return s

            xr_t = make_mix(3, "mixt")
            proj_cm(Wp["r"], sink_a2a(0), lambda k: xr_t[:, k, :])
            xk_t = make_mix(1, "kfB")
            proj_cm(Wp["k"], sink_a2a(1), lambda k: xk_t[:, k, :])

            # w decay
            xw_t = make_mix(0, "mixt")
            t1p = pp(64, TB)
            for k in range(KC):
                nc.tensor.matmul(t1p[:], c_td1[:, k, :], xw_t[:, k, :],
                                 start=(k == 0), stop=(k == KC - 1))
            t1 = cpool.tile([64, TB], BF16, tag="t1")
            nc.scalar.activation(t1[:], t1p[:], AF.Tanh)
            for m in range(KC):
                wp2 = pp(128, TB)
                nc.tensor.matmul(wp2[:], c_td2[:, m, :], t1[:],
                                 start=True, stop=True)
                st = sc.tile([128, TB], F32, tag="g2")
                nc.vector.tensor_scalar(st[:], wp2[:], c_td[:, m, 0:1], None,
                                        ALU.add)
                nc.sync.dma_start(
                    a2a_in[m // 2, 2, 128 * (m % 2):128 * (m % 2) + 128, :],
                    st[:])

            # v (token-major out)
            xv_t = make_mix(2, "kfB")
            for cc in range(4):
                pvs = [pp(128, TB) for _ in range(4)]
                for k in range(KC):
                    wv_t = wstr.tile([128, TB], BF16, tag="wv_s")
                    nc.sync.dma_start(
                        wv_t[:], Wv[128 * k:128 * (k + 1),
                                    512 * cc:512 * (cc + 1)])
                    for t4 in range(4):
                        nc.tensor.matmul(
                            pvs[t4][:], xv_t[:, k, 128 * t4:128 * (t4 + 1)],
                            wv_t[:], start=(k == 0), stop=(k == KC - 1))
                for t4 in range(4):
                    st = sc.tile([128, TB], F32, tag="g2")
                    nc.vector.tensor_copy(st[:], pvs[t4][:])
                    for half in range(2):
                        nc.sync.dma_start(
                            a2v_in[2 * cc + half, 128 * t4:128 * (t4 + 1), :],
                            st[:, 256 * half:256 * (half + 1)])

            # ============ A2A forward ============
            nc.gpsimd.collective_compute(
                "AllToAll", ALU.bypass, replica_groups=RG,
                ins=[a2a_in[:]], outs=[a2a_out[:]])
            nc.gpsimd.collective_compute(
                "AllToAll", ALU.bypass, replica_groups=RG,
                ins=[a2v_in[:]], outs=[a2v_out[:]])

            # g projection (overlaps collectives / WKV)
            xg_t = make_mix(4, "mixt")

            def sink_g(m, pt):
                nc.scalar.activation(gsb[:, m, :], pt[:], AF.Silu)
            proj_cm(Wp["g"], sink_g, lambda k: xg_t[:, k, :])

            # ============ WKV ============
            for hp in range(2):
                for b in range(2):
                    S2 = cpool.tile([128, 64], F32, tag=f"S_{hp}_{b}")
                    nc.vector.memset(S2[:], 0.0)
                    for jb in range(4):
                        j = 4 * b + jb
                        hs = slice(128 * hp, 128 * (hp + 1))
                        r2 = scw.tile([128, TB], F32, tag="wkv_r", bufs=2)
                        k2 = scw.tile([128, TB], F32, tag="wkv_k", bufs=2)
                        w2 = scw.tile([128, TB], F32, tag="wkv_w", bufs=2)
                        v2 = scw.tile([128, 4, 128], F32, tag="wkv_v", bufs=2)
                        nc.sync.dma_start(r2[:], a2a_out[j, 0, hs, :])
                        nc.sync.dma_start(k2[:], a2a_out[j, 1, hs, :])
                        nc.sync.dma_start(w2[:], a2a_out[j, 2, hs, :])
                        nc.sync.dma_start(
                            v2[:], a2v_out[j, :, hs]
                            .rearrange("(cc p) c -> p cc c", p=128))
                        e = scw.tile([128, TB], F32, tag="wkv_e")
                        nc.scalar.activation(e[:], w2[:], AF.Exp)
                        qe = scw.tile([128, TB], F32, tag="wkv_qe")
                        for cc in range(4):
                            cs = slice(128 * cc, 128 * (cc + 1))
                            nc.vector.tensor_tensor_scan(
                                qe[:, cs], e[:, cs], e[:, cs], 0.0,
                                ALU.add, ALU.bypass)
                        ku = scw.tile([128, TB], F32, tag="wkv_ku")
                        nc.vector.tensor_scalar(ku[:], k2[:],
                                                c_u[:, hp:hp + 1], None,
                                                ALU.mult)
                        e2f = scw.tile([128, TB], F32, tag="wkv_e2f")
                        nc.vector.tensor_mul(e2f[:], r2[:], ku[:])
                        # rt = r*exp(e-qe) (in place)
                        nc.vector.tensor_sub(e[:], e[:], qe[:])
                        nc.scalar.activation(e[:], e[:], AF.Exp)
                        nc.vector.tensor_mul(r2[:], r2[:], e[:])
                        # kt = k*exp(qe) (in place)
                        ktt = scw.tile([128, TB], F32, tag="wkv_e")
                        nc.scalar.activation(ktt[:], qe[:], AF.Exp)
                        nc.vector.tensor_mul(k2[:], k2[:], ktt[:])
                        ypb = sc.tile([128, TB], F32, tag="wkv_ypTs",
                                      bufs=1, name="ypb")
                        for cc in range(4):
                            cs = slice(128 * cc, 128 * (cc + 1))
                            qend = qe[:, 128 * cc + 127:128 * cc + 128]
                            pl2 = sc.tile([128, 1], F32, tag="wkv_pl")
                            nc.scalar.activation(pl2[:], qend, AF.Exp,
                                                 scale=-1.0)
                            kh = sc.tile([128, 128], F32, tag="wkv_kh")
                            nc.vector.tensor_scalar(kh[:], k2[:, cs], pl2[:],
                                                    None, ALU.mult)
                            khT = pp(128, 128)
                            nc.tensor.transpose(khT[:], kh[:], c_id[:])
                            khTs = sc.tile([128, 128], F32, tag="wkv_khTs")
                            nc.vector.tensor_copy(khTs[:], khT[:])
                            ypk = sc.tile([128, 128], F32, tag="wkv_ypk")
                            for hh in range(2):
                                h64 = slice(64 * hh, 64 * (hh + 1))
                                at = pp(128, 128)
                                nc.tensor.matmul(at[:], k2[h64, cs],
                                                 r2[h64, cs],
                                                 start=True, stop=True)
                                scol = pp(128, 1)
                                nc.tensor.matmul(scol[:], e2f[h64, cs],
                                                 ones_col[h64, :],
                                                 start=True, stop=True)
                                am = sc.tile([128, 128], F32, tag="wkv_am")
                                nc.vector.tensor_mul(am[:], at[:], c_msk[:])
                                nc.vector.scalar_tensor_tensor(
                                    am[:], c_id[:], scol[:], am[:],
                                    ALU.mult, ALU.add)
                                yp = pp(128, 64)
                                nc.tensor.matmul(yp[:], am[:], v2[:, cc, h64],
                                                 start=True, stop=False)
                                nc.tensor.matmul(yp[:], r2[h64, cs],
                                                 S2[h64, :],
                                                 start=False, stop=True)
                                sps = pp(64, 64)
                                nc.tensor.matmul(sps[:], khTs[:, h64],
                                                 v2[:, cc, h64],
                                                 start=True, stop=True)
                                nc.vector.scalar_tensor_tensor(
                                    S2[h64, :], S2[h64, :], pl2[h64, :],
                                    sps[:], ALU.mult, ALU.add)
                                # group norm (token-major)
                                ysb = sc.tile([128, 64], F32, tag="wkv_ysb")
                                nc.vector.tensor_copy(ysb[:], yp[:])
                                ysq = sc.tile([128, 64], F32, tag="wkv_ysq")
                                st4 = sc.tile([128, 4], F32, tag="wkv_st")
                                nc.scalar.activation(ysq[:], ysb[:], AF.Square,
                                                     accum_out=st4[:, 1:2])
                                nc.vector.tensor_reduce(
                                    st4[:, 0:1], ysb[:], mybir.AxisListType.X,
                                    ALU.add)
                                nc.vector.tensor_scalar(
                                    st4[:, 2:3], st4[:, 0:1], 1.0 / 64, None,
                                    ALU.mult)
                                nc.vector.tensor_scalar(
                                    st4[:, 1:2], st4[:, 1:2], 1.0 / 64, None,
                                    ALU.mult)
                                nc.vector.tensor_mul(st4[:, 3:4], st4[:, 2:3],
                                                     st4[:, 2:3])
                                nc.vector.tensor_sub(st4[:, 3:4], st4[:, 1:2],
                                                     st4[:, 3:4])
                                nc.scalar.activation(st4[:, 3:4], st4[:, 3:4],
                                                     AF.Sqrt, bias=EPS_LNX)
                                nc.vector.reciprocal(st4[:, 3:4], st4[:, 3:4])
                                nc.vector.tensor_scalar(
                                    ysb[:], ysb[:], st4[:, 2:3], st4[:, 3:4],
                                    ALU.subtract, ALU.mult)
                                lw = 64 * (2 * hp + hh)
                                nc.vector.tensor_mul(ypk[:, h64], ysb[:],
                                                     c_lnx[:, lw:lw + 64])
                                nc.vector.tensor_add(
                                    ypk[:, h64], ypk[:, h64],
                                    c_lnx[:, LCH + lw:LCH + lw + 64])
                            ypT = pp(128, 128)
                            nc.tensor.transpose(ypT[:], ypk[:], c_id[:])
                            nc.vector.tensor_copy(ypb[:, cs], ypT[:])
                        nc.sync.dma_start(a2b_in[j, hs, :], ypb[:])

            # ============ A2A back ============
            nc.gpsimd.collective_compute(
                "AllToAll", ALU.bypass, replica_groups=RG,
                ins=[a2b_in[:]], outs=[a2b_out[:]])

            # ============ att + residual ============
            yat = big.tile([128, KC, TB], BF16, tag="mixt", name="yat")
            for m in range(KC):
                yt = sc.tile([128, TB], F32, tag="g1")
                nc.sync.dma_start(
                    yt[:],
                    a2b_out[m // 2, 128 * (m % 2):128 * (m % 2) + 128, :])
                nc.vector.tensor_mul(yat[:, m, :], yt[:], gsb[:, m, :])

            for m in range(KC):
                wt = wstr.tile([128, KC, 128], BF16, tag="wstream", bufs=3)
                nc.sync.dma_start(
                    wt[:], Wp["o"][m * C:(m + 1) * C, :]
                    .rearrange("(k p) f -> p k f", p=128))
                pt = pp(128, TB)
                for k in range(KC):
                    nc.tensor.matmul(pt[:], wt[:, k, :], yat[:, k, :],
                                     start=(k == 0), stop=(k == KC - 1))
                x2t = sc.tile([128, TB], F32, tag="g3", bufs=1)
                xin = sc.tile([128, TB], F32, tag="g4", bufs=1)
                nc.sync.dma_start(xin[:], xT[128 * m:128 * (m + 1), 1:TB + 1])
                nc.vector.tensor_add(x2t[:], pt[:], xin[:])
                nc.sync.dma_start(x2d[128 * m:128 * (m + 1), 1:TB + 1], x2t[:])

            # ---- ln2 main pass over own 512 tokens (no AG dependency) ----
            psA3, psA4 = pp(1, TB), pp(1, TB)

            def x2_src(k):
                t = sc.tile([128, TB], F32, tag="e1")
                nc.sync.dma_start(t[:], x2d[128 * k:128 * (k + 1), 1:TB + 1])
                return t

            for k in range(KC):
                s = x2_src(k)
                sq = sc.tile([128, TB], F32, tag="e2")
                nc.scalar.activation(sq[:], s[:], AF.Square)
                st, sp = (k == 0), (k == KC - 1)
                nc.tensor.matmul(psA3[:], ones_col[:], s[:], start=st, stop=sp)
                nc.tensor.matmul(psA4[:], ones_col[:], sq[:], start=st, stop=sp)
            stats2 = lnp.tile([1, 2 * TB], F32, tag="ln_stats")
            mean2, msq2 = stats2[:, 0:TB], stats2[:, TB:]
            nc.scalar.activation(mean2[:], psA3[:], AF.Copy, scale=1.0 / C)
            nc.scalar.activation(msq2[:], psA4[:], AF.Copy, scale=1.0 / C)
            wk2 = lnp.tile([1, TB], F32, tag="ln_work")
            nc.vector.tensor_mul(wk2[:], mean2[:], mean2[:])
            nc.vector.tensor_sub(wk2[:], msq2[:], wk2[:])
            nc.scalar.activation(wk2[:], wk2[:], AF.Sqrt, bias=EPS_LN)
            nc.vector.reciprocal(wk2[:], wk2[:])
            bmp3, bip3 = pp(128, TB), pp(128, TB)
            nc.tensor.matmul(bmp3[:], ones_row[:], mean2[:], start=True,
                             stop=True)
            nc.tensor.matmul(bip3[:], ones_row[:], wk2[:], start=True,
                             stop=True)
            bc2 = lnp.tile([128, 2 * TB], BF16, tag="ln_bc")
            nc.vector.tensor_copy(bc2[:, 0:TB], bmp3[:])
            nc.vector.tensor_copy(bc2[:, TB:], bip3[:])
            for k in range(KC):
                s = x2_src(k)
                t = sc.tile([128, TB], F32, tag="e2")
                nc.vector.tensor_sub(t[:], s[:], bc2[:, 0:TB])
                nc.vector.tensor_mul(t[:], t[:], bc2[:, TB:])
                nc.vector.tensor_scalar(ht[:, k, 1:TB + 1], t[:],
                                        c_ln2[:, k, 0:1], c_ln2[:, k, 1:2],
                                        ALU.mult, ALU.add)
                # h2 boundary (own last token) -> ag_in for the neighbor
                nc.sync.dma_start(ag_in[0:1, 128 * k:128 * (k + 1)],
                                  ht[:, k, TB:TB + 1])

            nc.gpsimd.collective_compute(
                "AllGather", ALU.bypass, replica_groups=RG,
                ins=[ag_in[:]], outs=[ag_out[:]])

            for q in range(4):
                agp = sc.tile([NCORE, TB], BF16, tag="agp", bufs=1)
                nc.sync.dma_start(agp[:], ag_out[:, 512 * q:512 * (q + 1)])
                hp_ = pp(1, TB)
                nc.tensor.matmul(hp_[:], c_sel[:], agp[:],
                                 start=True, stop=True)
                hrow = sc.tile([1, TB], BF16, tag="hrow")
                nc.vector.tensor_copy(hrow[:], hp_[:])
                for mm in range(4):
                    m = 4 * q + mm
                    nc.sync.dma_start(ht[:, m, 0:1],
                                      hrow[0:1, 128 * mm:128 * (mm + 1)])

            for k in range(KC):
                xx2 = sc.tile([128, TB], F32, tag="g1")
                nc.vector.tensor_sub(xx2[:], ht[:, k, 0:TB],
                                     ht[:, k, 1:TB + 1])
                nc.vector.scalar_tensor_tensor(
                    xx[:, k, :], xx2[:], c_cm[:, k, 0:1], ht[:, k, 1:TB + 1],
                    ALU.mult, ALU.add)        # xk2
                nc.vector.scalar_tensor_tensor(
                    gsb[:, k, :], xx2[:], c_cm[:, k, 1:2], ht[:, k, 1:TB + 1],
                    ALU.mult, ALU.add)        # xr2

            # ============ FFN ============
            kfA = big.tile([128, KC, TB], BF16, tag="mixt")  # alias mixt slot
            kfB = big.tile([128, 32, TB], BF16, tag="kfB")
            kfC = big.tile([128, 8, TB], BF16, tag="ht")     # alias ht slot

            def kf_view(i):
                if i < KC:
                    return kfA[:, i, :]
                return kfB[:, i - KC, :] if i < 48 else kfC[:, i - 48, :]

            for mf in range(KF):
                wt = wstr.tile([128, KC, 128], BF16, tag="wstream", bufs=3)
                nc.sync.dma_start(
                    wt[:], Wck_p[mf * C:(mf + 1) * C, :]
                    .rearrange("(k p) f -> p k f", p=128))
                pt = pp(128, TB)
                for k in range(KC):
                    nc.tensor.matmul(pt[:], wt[:, k, :], xx[:, k, :],
                                     start=(k == 0), stop=(k == KC - 1))
                rl = sc.tile([128, TB], F32, tag="g1")
                nc.vector.tensor_scalar(rl[:], pt[:], 0.0, None, ALU.max)
                nc.scalar.activation(kf_view(mf), rl[:], AF.Square)

            for m in range(KC):
                ptu = pp(128, TB)
                for q in range(4):
                    wcv = wstr.tile([128, 14, 128], BF16, tag="wcv_s", bufs=2)
                    nc.sync.dma_start(
                        wcv[:], Wcv_p[m * FF + q * 14 * 128:
                                      m * FF + (q + 1) * 14 * 128, :]
                        .rearrange("(k p) f -> p k f", p=128))
                    for kk in range(14):
                        ki = q * 14 + kk
                        nc.tensor.matmul(ptu[:], wcv[:, kk, :], kf_view(ki),
                                         start=(ki == 0), stop=(ki == KF - 1))
                wt = wstr.tile([128, KC, 128], BF16, tag="wstream", bufs=3)
                nc.sync.dma_start(
                    wt[:], Wp["cr"][m * C:(m + 1) * C, :]
                    .rearrange("(k p) f -> p k f", p=128))
                pts = pp(128, TB)
                for k in range(KC):
                    nc.tensor.matmul(pts[:], wt[:, k, :], gsb[:, k, :],
                                     start=(k == 0), stop=(k == KC - 1))
                ssb = sc.tile([128, TB], F32, tag="g2")
                nc.scalar.activation(ssb[:], pts[:], AF.Sigmoid)
                ot = sc.tile([128, TB], F32, tag="g3", bufs=1)
                nc.vector.tensor_mul(ot[:], ptu[:], ssb[:])
                x2in = sc.tile([128, TB], F32, tag="g4", bufs=1)
                nc.sync.dma_start(x2in[:],
                                  x2d[128 * m:128 * (m + 1), 1:TB + 1])
                nc.vector.tensor_add(ot[:], ot[:], x2in[:])
                nc.sync.dma_start(outT[128 * m:128 * (m + 1), :], ot[:])

    nc.compile()
    return nc


_CACHE = {}


def _get_program():
    if "nc" not in _CACHE:
        _CACHE["nc"] = build_program()
    return _CACHE["nc"]


def _pret(w, mtile=128):
    ci, co = w.shape
    nb = co // mtile
    return np.ascontiguousarray(
        w.reshape(ci, nb, mtile).transpose(1, 0, 2)).reshape(nb * ci, mtile)


def _shard_inputs(inp):
    f32 = np.float32
    x = np.asarray(inp["x"], f32)
    bf = lambda a: np.asarray(a, f32).astype(NP_BF16)

    maa_w2 = np.asarray(inp["maa_w2"], f32)
    w2p = np.zeros((32, 5, KC, 128), f32)
    for i in range(5):
        for m in range(KC):
            w2p[:, i, m, :] = maa_w2[i][:, 128 * m:128 * (m + 1)]
    td_w2 = np.asarray(inp["td_w2"], f32)
    td2p = td_w2.reshape(64, KC, 128).copy()
    for m in range(KC):
        td2p[:, m, :] = td_w2[:, 128 * m:128 * (m + 1)]

    shared = {
        "ln1_wb": np.stack([inp["ln1_w"], inp["ln1_b"]], 1).astype(f32),
        "ln2_wb": np.stack([inp["ln2_w"], inp["ln2_b"]], 1).astype(f32),
        "tm_maaT": np.asarray(inp["tm_maa"], f32).T.copy(),
        "cm_maaT": np.asarray(inp["cm_maa"], f32).T.copy(),
        "td_col": np.asarray(inp["time_decay"], f32).reshape(C, 1),
        "ident": np.eye(128, dtype=f32),
        "mask_su": np.triu(np.ones((128, 128), f32), 1),
        "maa_w1": bf(inp["maa_w1"]),
        "maa_w2p": w2p.astype(NP_BF16),
        "td_w1": bf(inp["td_w1"]),
        "td_w2p": td2p.astype(NP_BF16),
        "Wr_p": bf(_pret(np.asarray(inp["Wr"], f32))),
        "Wk_p": bf(_pret(np.asarray(inp["Wk"], f32))),
        "Wg_p": bf(_pret(np.asarray(inp["Wg"], f32))),
        "Wo_p": bf(_pret(np.asarray(inp["Wo"], f32))),
        "Wcr_p": bf(_pret(np.asarray(inp["Wcr"], f32))),
        "Wv": bf(inp["Wv"]),
        "Wck_p": bf(_pret(np.asarray(inp["Wck"], f32))),
        "Wcv_p": bf(_pret(np.asarray(inp["Wcv"], f32))),
    }
    u = np.asarray(inp["time_faaaa"], f32).reshape(C)
    lnx_w = np.asarray(inp["lnx_w"], f32)
    lnx_b = np.asarray(inp["lnx_b"], f32)

    in_maps = []
    for c in range(NCORE):
        b, blk = c // 4, c % 4
        ts = blk * TB
        xe = np.zeros((C, TB + 1), f32)
        xe[:, 1:] = x[b, ts:ts + TB].T
        if blk > 0:
            xe[:, 0] = x[b, ts - 1]
        ul = u[LCH * c:LCH * (c + 1)].reshape(2, 128).T.copy()
        lw = np.tile(lnx_w[LCH * c:LCH * (c + 1)][None, :], (128, 1))
        lb = np.tile(lnx_b[LCH * c:LCH * (c + 1)][None, :], (128, 1))
        sel = np.zeros((NCORE, 1), NP_BF16)
        if blk > 0:
            sel[c - 1, 0] = 1.0
        m = dict(shared)
        m.update({
            "xT": xe,
            "halo_mask": np.full((128, 1), 1.0 if blk > 0 else 0.0, f32),
            "sel_prev": sel,
            "u_loc": ul,
            "lnx_bc": np.concatenate([lw, lb], 1).astype(f32),
        })
        in_maps.append(m)
    return in_maps


def run(inputs, trace=False):
    nc = _get_program()
    in_maps = _shard_inputs(inputs)
    res = bass_utils.run_bass_kernel_spmd(
        nc, in_maps, core_ids=list(range(NCORE)), trace=trace)
    x = np.asarray(inputs["x"], np.float32)
    out = np.empty_like(x)
    for c in range(NCORE):
        b, blk = c // 4, c % 4
        out[b, blk * TB:(blk + 1) * TB, :] = np.asarray(
            res.results[c]["out"], np.float32).T
    return out, res.exec_time_ns


def kernel(**inputs):
    out, _ = run(inputs)
    return out


if __name__ == "__main__":
    build_program()
    print("build ok")

